# revision 31
# baseline (speedup 1.0000x reference)
"""Trainium2 Bass kernel for nn_ChainOfExperts (MoE with shared experts).

Strategy (8 NeuronCores):
  Phase 1 (data-parallel, tokens sharded along B): router logits + top-2
    softmax weights on-device, shared-expert FFN, and a bf16 cast of x.
  Host: pure data movement — group token slots by routed expert (counting
    sort on device-computed indices), gather bf16 token columns per expert.
  Phase 2 (expert-parallel, 2 experts per core): routed-expert FFN on the
    gathered tokens (capacity-padded), output pre-scaled by combine weight.
  Phase 3 (data-parallel): out = shared + y_slot0 + y_slot1.

All activations are kept feature-major ([D, tokens]) so every matmul has its
contraction dim on partitions. Matmuls run in bf16 (fp32 accumulate); the
router runs in fp32.
"""

import os
from contextlib import ExitStack
from dataclasses import dataclass

import numpy as np
import ml_dtypes

import concourse.bass as bass
import concourse.mybir as mybir
import concourse.tile as tile
from concourse import bacc
from concourse.bass import ts, ds, broadcast_tensor_aps
from concourse.bass_utils import run_bass_kernel_spmd
from concourse.kernels.tile_matmul import (
    ShapeInfo,
    composable_matmul_tile_kernel,
    cast_to_type,
    dma_from_dram_kxm,
    dma_from_dram_kxn,
    dma_to_dram_mxn,
    _tiled_ap,
)


def dma_to_dram_mxn_on(engine_name: str, ap):
    """dma_to_dram_mxn variant issuing on a chosen engine, spreading DMA
    descriptor generation off the (busy) sync sequencer."""
    ap, shape = _tiled_ap(ap)

    def consume(nc_, mxn_tile, md):
        n_sz = min(md.n_tile, shape.fdims[0] - md.n_tile_idx * md.n_tile)
        getattr(nc_, engine_name).dma_start(
            ap[
                :, ts(md.m_tile_idx, md.m_subtiles),
                ds(md.n_tile_idx * md.n_tile, n_sz),
            ],
            mxn_tile[:, :, :n_sz],
        )

    return consume

BF16 = ml_dtypes.bfloat16
ActFn = mybir.ActivationFunctionType
dt = mybir.dt
P = 128

# bass_utils imports antenv.axon_hooks when tracing is requested; this
# container ships only an antenv stub. Provide the missing module, wired
# to the axon ctypes NTFF hook when the injected .so supports it, so a
# trace request yields real device profiles instead of crashing.
import sys as _sys
try:
    import antenv.axon_hooks  # noqa: F401
except ImportError:
    import types as _types
    import antenv as _antenv
    _stub = _types.ModuleType("antenv.axon_hooks")
    _hook_box = [None]
    _stub.get_axon_ntff_profile_hook = lambda: _hook_box[0]
    _stub.set_axon_ntff_profile_hook = lambda h: _hook_box.__setitem__(0, h)
    _sys.modules["antenv.axon_hooks"] = _stub
    _antenv.axon_hooks = _stub
    try:
        from trn_agent_boot.trn_boot import _ntff_profile_via_ctypes
        _hook_box[0] = _ntff_profile_via_ctypes("/opt/axon/libaxon_pjrt.so")
    except Exception:
        pass


@dataclass(frozen=True)
class Cfg:
    n_cores: int = 8
    D: int = 2048     # hidden dim
    TPC: int = 2048   # tokens per core
    E: int = 16       # routed experts
    NSH: int = 2      # shared experts
    DS: int = 1024    # shared inner dim
    DR: int = 512     # routed inner dim
    CAP: int = 2304   # per-expert token capacity (multiple of 128)
    EPC: int = 2      # experts per core

    @property
    def n_tile(self):  # composable's N tiling for N=CAP
        return min(512, -(-self.CAP // P) * P)

    @property
    def n_tiles(self):
        return -(-self.CAP // self.n_tile)

    @property
    def CAPP(self):  # hg cache width: CAP padded to whole n-tiles
        return self.n_tiles * self.n_tile


CFG = Cfg()
TRACE = bool(os.environ.get("KERNEL_TRACE"))
LAST_EXEC_NS: dict[str, int | None] = {}

_cache: dict = {}


def _rearr2(ap):
    """[K, N] dram AP -> [pi, po, N] with K = po*128 + pi."""
    return ap.rearrange("(po pi) t -> pi po t", pi=P)


# --------------------------------------------------------------------------
# Phase 1: router + shared experts + bf16 cast of x
# --------------------------------------------------------------------------

class _NullCtx:
    def __enter__(self):
        return None

    def __exit__(self, *a):
        return False


def _maybe_loop(tc, loop_n):
    """Wrap the phase body in an in-NEFF repeat loop (for benchmarking)."""
    return tc.For_i(0, loop_n, 1) if loop_n else _NullCtx()


def build_p1(cfg: Cfg, debug: bool = False, loop_n: int = 0):
    nc = bacc.Bacc("TRN2", target_bir_lowering=False, debug=debug)
    f32 = dt.float32
    xT = nc.dram_tensor("xT", [cfg.D, cfg.TPC], f32, kind="ExternalInput").ap()
    rw = nc.dram_tensor("rw", [cfg.D, cfg.E], f32, kind="ExternalInput").ap()
    sw1 = nc.dram_tensor("sw1", [cfg.NSH, cfg.D, cfg.DS], f32, kind="ExternalInput").ap()
    sb1 = nc.dram_tensor("sb1", [cfg.NSH, cfg.DS], f32, kind="ExternalInput").ap()
    sw2 = nc.dram_tensor("sw2", [cfg.NSH, cfg.DS, cfg.D], f32, kind="ExternalInput").ap()
    sb2 = nc.dram_tensor("sb2", [cfg.NSH, cfg.D], f32, kind="ExternalInput").ap()
    out_shT = nc.dram_tensor("out_shT", [cfg.D, cfg.TPC], f32, kind="ExternalOutput").ap()
    xbfT = nc.dram_tensor("xbfT", [cfg.D, cfg.TPC], dt.bfloat16, kind="ExternalOutput").ap()
    ridx = nc.dram_tensor("ridx", [cfg.TPC, 8], dt.uint32, kind="ExternalOutput").ap()
    rwts = nc.dram_tensor("rwts", [cfg.TPC, 8], f32, kind="ExternalOutput").ap()
    h_dram = nc.dram_tensor("h_mid", [cfg.NSH, cfg.DS, cfg.TPC], dt.bfloat16).ap()

    x_po = cfg.D // P
    ds_po = cfg.DS // P
    CH = 256  # router/cast chunk (tokens)

    with tile.TileContext(nc) as tc, _maybe_loop(tc, loop_n), ExitStack() as ctx:
        const = ctx.enter_context(tc.tile_pool(name="const", bufs=1))
        rw_sb = const.tile([P, x_po, cfg.E], f32)
        nc.sync.dma_start(rw_sb[:], rw.rearrange("(po pi) e -> pi po e", pi=P))
        b1_sb = const.tile([P, cfg.NSH, ds_po], f32)
        nc.sync.dma_start(b1_sb[:], sb1.rearrange("s (po pi) -> pi s po", pi=P))
        b2_sb = const.tile([P, cfg.NSH, x_po], f32)
        nc.sync.dma_start(b2_sb[:], sb2.rearrange("s (po pi) -> pi s po", pi=P))
        b2sum = const.tile([P, x_po], f32)
        nc.vector.tensor_add(b2sum[:], b2_sb[:, 0], b2_sb[:, 1])
        xbf_cache = const.tile([P, x_po, cfg.TPC], dt.bfloat16)

        # ---- router + cast pass ----
        with ExitStack() as c2:
            xsrc = c2.enter_context(tc.tile_pool(name="xsrc", bufs=2))
            rps = c2.enter_context(tc.tile_pool(name="rpsum", bufs=2, space="PSUM"))
            rsb = c2.enter_context(tc.tile_pool(name="rsb", bufs=3))
            xT_t = _rearr2(xT)
            xbfT_t = _rearr2(xbfT)
            for c in range(cfg.TPC // CH):
                xt = xsrc.tile([P, x_po, CH], f32, tag="xt")
                nc.sync.dma_start(xt[:], xT_t[:, :, ts(c, CH)])
                nc.vector.tensor_copy(xbf_cache[:, :, ts(c, CH)], xt[:])
                nc.sync.dma_start(xbfT_t[:, :, ts(c, CH)], xbf_cache[:, :, ts(c, CH)])
                for tt in range(CH // P):
                    t0 = c * CH + tt * P
                    ps = rps.tile([P, cfg.E], f32, tag="rp")
                    for po in range(x_po):
                        nc.tensor.matmul(
                            ps[:], xt[:, po, ts(tt, P)], rw_sb[:, po, :],
                            start=(po == 0), stop=(po == x_po - 1),
                        )
                    lg = rsb.tile([P, cfg.E], f32, tag="lg")
                    nc.vector.tensor_copy(lg[:], ps[:])
                    mx = rsb.tile([P, 8], f32, tag="mx")
                    nc.vector.max(mx[:], lg[:])
                    ix = rsb.tile([P, 8], dt.uint32, tag="ix")
                    nc.vector.max_index(ix[:], mx[:], lg[:])
                    nm = rsb.tile([P, 1], f32, tag="nm")
                    nc.vector.tensor_scalar_mul(nm[:], mx[:, 0:1], -1.0)
                    ex = rsb.tile([P, cfg.E], f32, tag="ex")
                    zz = rsb.tile([P, 1], f32, tag="zz")
                    nc.scalar.activation(ex[:], lg[:], ActFn.Exp, bias=nm[:], accum_out=zz[:])
                    rz = rsb.tile([P, 1], f32, tag="rz")
                    nc.vector.reciprocal(rz[:], zz[:])
                    wv = rsb.tile([P, 8], f32, tag="wv")
                    nc.scalar.activation(wv[:], mx[:], ActFn.Exp, bias=nm[:])
                    nc.vector.tensor_scalar_mul(wv[:], wv[:], rz[:])
                    nc.sync.dma_start(ridx[ds(t0, P), :], ix[:])
                    nc.sync.dma_start(rwts[ds(t0, P), :], wv[:])

        # ---- shared experts layer 1 (per shared expert s) ----
        xbf_shape = ShapeInfo(pdims=((P, x_po),), fdims=(cfg.TPC,))

        def xbf_producer(nc_, md):
            return xbf_cache[
                :, ts(md.k_tile_idx, md.k_subtiles),
                ds(md.n_tile_idx * md.n_tile, md.n_tile)
            ]

        for s in range(cfg.NSH):
            with ExitStack() as c2:
                mpool = c2.enter_context(tc.tile_pool(name=f"l1m{s}", bufs=2))
                cpool = c2.enter_context(tc.tile_pool(name=f"l1c{s}", bufs=5))
                kxm_prod, kxm_shape = dma_from_dram_kxm(mpool, sw1[s])
                kxm_prod = cast_to_type(kxm_prod, cpool, dt.bfloat16)

                def l1_reducer(nc_, psum, sbuf, md, s=s):
                    ko = (md.m_tile_idx * md.m_tile + md.m_subtile_idx * P) // P
                    nc_.scalar.activation(
                        sbuf[:], psum[:], ActFn.Silu, bias=b1_sb[:, s, ko:ko + 1]
                    )

                composable_matmul_tile_kernel(
                    tc=tc,
                    kxm_shape=kxm_shape,
                    kxn_shape=xbf_shape,
                    output_type=dt.bfloat16,
                    kxm_producer=kxm_prod,
                    kxn_producer=xbf_producer,
                    mxn_consumer=dma_to_dram_mxn(h_dram[s]),
                    mxn_subtile_reducer=l1_reducer,
                )

        # ---- shared experts layer 2 (contract over s and DS jointly) ----
        with ExitStack() as c2:
            mpool = c2.enter_context(tc.tile_pool(name="l2m", bufs=2))
            cpool = c2.enter_context(tc.tile_pool(name="l2c", bufs=5))
            npool = c2.enter_context(tc.tile_pool(name="l2n", bufs=5))
            kxm_prod, kxm_shape = dma_from_dram_kxm(mpool, sw2, batch_k=True)
            kxm_prod = cast_to_type(kxm_prod, cpool, dt.bfloat16)
            kxn_prod, kxn_shape = dma_from_dram_kxn(npool, h_dram, batch_k=True)

            def l2_reducer(nc_, psum, sbuf, md):
                do = (md.m_tile_idx * md.m_tile + md.m_subtile_idx * P) // P
                nc_.vector.tensor_scalar_add(sbuf[:], psum[:], b2sum[:, do:do + 1])

            composable_matmul_tile_kernel(
                tc=tc,
                kxm_shape=kxm_shape,
                kxn_shape=kxn_shape,
                output_type=dt.float32,
                kxm_producer=kxm_prod,
                kxn_producer=kxn_prod,
                mxn_consumer=dma_to_dram_mxn(out_shT),
                mxn_subtile_reducer=l2_reducer,
            )

    nc.compile()
    return nc


def build_p1r(cfg: Cfg, debug: bool = False, loop_n: int = 0):
    """Router-only phase: top-2 indices/weights.

    Router matmuls keep rw stationary (16-col LDWEIGHTS, ~13ns) with x as
    the moving operand, accumulating transposed logits [E, TPC] in PSUM
    across all D subtiles. PE transposes then yield [128-token, E] groups
    for a top-2 softmax batched across all groups (one DVE op per step).
    """
    nc = bacc.Bacc("TRN2", target_bir_lowering=False, debug=debug)
    f32 = dt.float32
    E = cfg.E
    xT = nc.dram_tensor("xT", [cfg.D, cfg.TPC], f32, kind="ExternalInput").ap()
    rw = nc.dram_tensor("rw", [cfg.D, E], f32, kind="ExternalInput").ap()
    ident = nc.dram_tensor("ident", [E, E], f32, kind="ExternalInput").ap()
    ridx = nc.dram_tensor("ridx", [cfg.TPC, 8], dt.uint32, kind="ExternalOutput").ap()
    rwts = nc.dram_tensor("rwts", [cfg.TPC, 8], f32, kind="ExternalOutput").ap()

    x_po = cfg.D // P
    PO_CH = 2
    NPO = x_po // PO_CH
    NG = cfg.TPC // P
    NT = cfg.TPC // 512

    with tile.TileContext(nc) as tc, _maybe_loop(tc, loop_n), ExitStack() as ctx:
        const = ctx.enter_context(tc.tile_pool(name="const", bufs=1))
        rw_sb = const.tile([P, x_po, E], f32)
        nc.sync.dma_start(rw_sb[:], rw.rearrange("(po pi) e -> pi po e", pi=P))
        id_sb = const.tile([E, E], f32)
        nc.sync.dma_start(id_sb[:], ident)

        xsrc = ctx.enter_context(tc.tile_pool(name="xsrc", bufs=4))
        lgps = ctx.enter_context(tc.tile_pool(name="lgps", bufs=1, space="PSUM"))
        tps = ctx.enter_context(tc.tile_pool(name="tps", bufs=1, space="PSUM"))
        sm = ctx.enter_context(tc.tile_pool(name="sm", bufs=1))
        xT_t = _rearr2(xT)

        lgT_ps = lgps.tile([E, cfg.TPC], f32)
        for po8 in range(NPO):
            xt = xsrc.tile([P, PO_CH, cfg.TPC], f32, tag="xt")
            nc.sync.dma_start(xt[:], xT_t[:, ts(po8, PO_CH), :])
            for pp in range(PO_CH):
                po = po8 * PO_CH + pp
                for t in range(NT):
                    nc.tensor.matmul(
                        lgT_ps[:, ts(t, 512)], rw_sb[:, po, :],
                        xt[:, pp, ts(t, 512)],
                        start=(po == 0), stop=(po == x_po - 1),
                        skip_group_check=True,
                    )

        lgT_sb = sm.tile([E, cfg.TPC], f32, tag="lgT")
        nc.vector.tensor_copy(lgT_sb[:], lgT_ps[:])
        lg_ps = tps.tile([P, NG, E], f32)
        for g in range(NG):
            nc.tensor.transpose(lg_ps[:, g, :], lgT_sb[:, ts(g, P)], id_sb[:])
        lg_all = sm.tile([P, NG, E], f32, tag="lg")
        nc.vector.tensor_copy(lg_all[:], lg_ps[:])

        mx = sm.tile([P, NG, 8], f32, tag="mx")
        ix = sm.tile([P, NG, 8], dt.uint32, tag="ix")
        for g in range(NG):
            nc.vector.max(mx[:, g, :], lg_all[:, g, :])
            nc.vector.max_index(ix[:, g, :], mx[:, g, :], lg_all[:, g, :])

        # batched stable softmax: nm = -top1; w_k = exp(m_k+nm)/sum(exp(lg+nm))
        nm = sm.tile([P, NG, 1], f32, tag="nm")
        nc.vector.tensor_scalar_mul(nm[:], mx[:, :, 0:1], -1.0)
        d8 = sm.tile([P, NG, 8], f32, tag="d8")
        a, b = broadcast_tensor_aps(mx[:], nm[:])
        nc.vector.tensor_add(d8[:], a, b)
        wv = sm.tile([P, NG, 8], f32, tag="wv")
        nc.scalar.activation(wv[:], d8[:], ActFn.Exp)
        dE = sm.tile([P, NG, E], f32, tag="dE")
        a, b = broadcast_tensor_aps(lg_all[:], nm[:])
        nc.vector.tensor_add(dE[:], a, b)
        ex = sm.tile([P, NG, E], f32, tag="ex")
        nc.scalar.activation(ex[:], dE[:], ActFn.Exp)
        zz = sm.tile([P, NG], f32, tag="zz")
        nc.vector.tensor_reduce(
            zz[:], ex[:], mybir.AxisListType.X, mybir.AluOpType.add
        )
        rz = sm.tile([P, NG, 1], f32, tag="rz")
        nc.vector.reciprocal(rz[:], zz[:])
        a, b = broadcast_tensor_aps(wv[:], rz[:])
        nc.vector.tensor_mul(wv[:], a, b)

        nc.sync.dma_start(ridx.rearrange("(g pi) k -> pi g k", pi=P), ix[:])
        nc.sync.dma_start(rwts.rearrange("(g pi) k -> pi g k", pi=P), wv[:])
    nc.compile()
    return nc


def _w_producer_batched(pool, w_ap, tagname):
    """Batched-K variant of _w_producer for [S, K, M] weights."""
    S, K, M = w_ap.shape
    shape = ShapeInfo(pdims=((P, K // P),) * S, fdims=(M,))
    w_ts = [w_ap[s].rearrange("(po pi) m -> pi po m", pi=P) for s in range(S)]
    cast = w_ap.dtype == dt.float32

    def prod(nc_, md):
        t = pool.tile([P, md.k_subtiles, md.m_tile],
                      dt.bfloat16 if cast else w_ap.dtype, tag=tagname)
        nc_.gpsimd.dma_start(
            t[:],
            w_ts[md.k_batch_idx][
                :, ts(md.k_tile_idx, md.k_subtiles),
                ds(md.m_tile_idx * md.m_tile, md.m_tile)
            ],
        )
        return t

    return prod, shape


def build_p3s(cfg: Cfg, debug: bool = False, loop_n: int = 0, has_b2: bool = False):
    """Shared experts + combine: out = sharedFFN(x) + ya + yb.

    h is kept in SBUF as per-(s, n-tile) tiles so layer 2 pipelines with
    layer 1 at tile granularity (no DRAM roundtrip, no coarse-dep stall).
    """
    nc = bacc.Bacc("TRN2", target_bir_lowering=False, debug=debug)
    f32 = dt.float32
    xT = nc.dram_tensor("xT", [cfg.D, cfg.TPC], f32, kind="ExternalInput").ap()
    # sw1 is the two shared-expert w1s concatenated on the inner dim (host)
    sw1 = nc.dram_tensor("sw1", [cfg.D, cfg.NSH * cfg.DS], dt.bfloat16, kind="ExternalInput").ap()
    sb1 = nc.dram_tensor("sb1", [cfg.NSH, cfg.DS], f32, kind="ExternalInput").ap()
    sw2 = nc.dram_tensor("sw2", [cfg.NSH, cfg.DS, cfg.D], dt.bfloat16, kind="ExternalInput").ap()
    sb2 = nc.dram_tensor("sb2", [cfg.NSH, cfg.D], f32, kind="ExternalInput").ap()
    yaT = nc.dram_tensor("yaT", [cfg.D, cfg.TPC], dt.bfloat16, kind="ExternalInput").ap()
    ybT = nc.dram_tensor("ybT", [cfg.D, cfg.TPC], dt.bfloat16, kind="ExternalInput").ap()
    outT = nc.dram_tensor("outT", [cfg.D, cfg.TPC], f32, kind="ExternalOutput").ap()

    x_po = cfg.D // P
    ds_po = cfg.DS // P
    NT = cfg.TPC // 512  # n tiles

    with tile.TileContext(nc) as tc, _maybe_loop(tc, loop_n), ExitStack() as ctx:
        const = ctx.enter_context(tc.tile_pool(name="const", bufs=1))
        b1_sb = const.tile([P, cfg.NSH, ds_po], f32)
        nc.sync.dma_start(b1_sb[:], sb1.rearrange("s (po pi) -> pi s po", pi=P))
        b2_sb = const.tile([P, cfg.NSH, x_po], f32)
        nc.sync.dma_start(b2_sb[:], sb2.rearrange("s (po pi) -> pi s po", pi=P))
        b2sum = const.tile([P, x_po], f32)
        nc.vector.tensor_add(b2sum[:], b2_sb[:, 0], b2_sb[:, 1])

        hpool = ctx.enter_context(tc.tile_pool(name="hp", bufs=1))
        h_tiles = [
            [
                hpool.tile([P, ds_po, 512], dt.bfloat16,
                           tag=f"h{s}_{n}", name=f"h{s}_{n}")
                for n in range(NT)
            ]
            for s in range(cfg.NSH)
        ]

        with ExitStack() as c2:
            mpool = c2.enter_context(tc.tile_pool(name="l1m", bufs=3))
            npool = c2.enter_context(tc.tile_pool(name="l1n", bufs=3))
            cpool = c2.enter_context(tc.tile_pool(name="l1c", bufs=3))
            kxm_prod, kxm_shape = _w_producer(mpool, sw1, "sw1")
            kxn_prod, kxn_shape = dma_from_dram_kxn(npool, xT)
            kxn_prod = cast_to_type(kxn_prod, cpool, dt.bfloat16)

            def l1_reducer(nc_, psum, sbuf, md):
                ko_abs = (md.m_tile_idx * md.m_tile + md.m_subtile_idx * P) // P
                s, ko = divmod(ko_abs, ds_po)
                nc_.scalar.activation(
                    sbuf[:], psum[:], ActFn.Silu, bias=b1_sb[:, s, ko:ko + 1]
                )

            def h_producer(nc_, md):
                ko_abs = (md.m_tile_idx * md.m_tile) // P
                s, off = divmod(ko_abs, ds_po)
                return h_tiles[s][md.n_tile_idx][
                    :, ds(off, md.m_tile // P), :
                ]

            composable_matmul_tile_kernel(
                tc=tc,
                kxm_shape=kxm_shape,
                kxn_shape=kxn_shape,
                output_type=None,
                kxm_producer=kxm_prod,
                kxn_producer=kxn_prod,
                mxn_consumer=lambda nc_, sbuf, md: None,
                mxn_subtile_reducer=l1_reducer,
                mxn_subtile_producer=h_producer,
                psum_n_bufs=2,
                MAX_K_TILE_SIZE=1024,
            )

        with ExitStack() as c2:
            mpool = c2.enter_context(tc.tile_pool(name="l2m", bufs=6))
            apool = c2.enter_context(tc.tile_pool(name="addp", bufs=3))
            kxm_prod, kxm_shape = _w_producer_batched(mpool, sw2, "sw2")
            kxn_shape = ShapeInfo(pdims=((P, ds_po),) * cfg.NSH, fdims=(cfg.TPC,))

            def h_kxn_producer(nc_, md):
                return h_tiles[md.k_batch_idx][md.n_tile_idx][
                    :, ts(md.k_tile_idx, md.k_subtiles), :
                ]

            def l2_reducer(nc_, psum, sbuf, md):
                do = md.m_tile_idx * (md.m_tile // P) + md.m_subtile_idx
                if has_b2:
                    nc_.vector.tensor_scalar_add(sbuf[:], psum[:], b2sum[:, do:do + 1])
                else:
                    nc_.vector.tensor_copy(sbuf[:], psum[:])

            base_consumer = dma_to_dram_mxn(outT)
            yaT_t, ybT_t = _rearr2(yaT), _rearr2(ybT)

            def combine_consumer(nc_, sbuf, md):
                po0 = md.m_tile_idx * (md.m_tile // P)
                nsub = md.m_tile // P
                nsl = ds(md.n_tile_idx * md.n_tile, md.n_tile)
                ya_t = apool.tile([P, nsub, md.n_tile], dt.bfloat16, tag="ya")
                nc_.scalar.dma_start(ya_t[:], yaT_t[:, ds(po0, nsub), nsl])
                yb_t = apool.tile([P, nsub, md.n_tile], dt.bfloat16, tag="yb")
                nc_.scalar.dma_start(yb_t[:], ybT_t[:, ds(po0, nsub), nsl])
                nc_.vector.tensor_add(sbuf[:], sbuf[:], ya_t[:])
                nc_.vector.tensor_add(sbuf[:], sbuf[:], yb_t[:])
                base_consumer(nc_, sbuf, md)

            composable_matmul_tile_kernel(
                tc=tc,
                kxm_shape=kxm_shape,
                kxn_shape=kxn_shape,
                output_type=dt.float32,
                kxm_producer=kxm_prod,
                kxn_producer=h_kxn_producer,
                mxn_consumer=combine_consumer,
                mxn_subtile_reducer=l2_reducer,
                psum_n_bufs=2,
                MAX_K_TILE_SIZE=1024,
            )
    nc.compile()
    return nc


# --------------------------------------------------------------------------
# Phase 2: routed experts (expert-parallel, capacity padded)
# --------------------------------------------------------------------------

def _w_producer(pool, w_ap, tagname):
    """kxm producer streaming a [K, M] weight from DRAM. f32 sources are
    cast to bf16 via SWDGE cast-DMA; bf16/fp8 sources stream as-is."""
    K, M = w_ap.shape
    shape = ShapeInfo(pdims=((P, K // P),), fdims=(M,))
    w_t = w_ap.rearrange("(po pi) m -> pi po m", pi=P)
    cast = w_ap.dtype == dt.float32

    def prod(nc_, md):
        t = pool.tile(
            [P, md.k_subtiles, md.m_tile],
            dt.bfloat16 if cast else w_ap.dtype, tag=tagname
        )
        nc_.gpsimd.dma_start(
            t[:],
            w_t[
                :, ts(md.k_tile_idx, md.k_subtiles),
                ds(md.m_tile_idx * md.m_tile, md.m_tile)
            ],
        )
        return t

    return prod, shape


def build_p2(cfg: Cfg, debug: bool = False, loop_n: int = 0, has_b2: bool = False):
    """Routed-expert FFN in fp8 (DoubleRow matmuls).

    Host pre-scales w1/w2 by 64 and casts to fp8e4; the 1/64 descale is
    folded into the silu activation (layer 1) and the combine weights
    (layer 2). x and h are fp8 at scale 1.
    """
    nc = bacc.Bacc("TRN2", target_bir_lowering=False, debug=debug)
    f32 = dt.float32
    f8 = dt.float8e4
    W = cfg.EPC * cfg.CAP
    xgT = nc.dram_tensor("xgT", [cfg.D, W], f8, kind="ExternalInput").ap()
    ew1 = nc.dram_tensor("ew1", [cfg.EPC, cfg.D, cfg.DR], f8, kind="ExternalInput").ap()
    eb1 = nc.dram_tensor("eb1", [cfg.EPC, cfg.DR], f32, kind="ExternalInput").ap()
    ew2 = nc.dram_tensor("ew2", [cfg.EPC, cfg.DR, cfg.D], f8, kind="ExternalInput").ap()
    eb2 = nc.dram_tensor("eb2", [cfg.EPC, cfg.D], f32, kind="ExternalInput").ap()
    cw = nc.dram_tensor("cw", [cfg.EPC, cfg.CAPP], f32, kind="ExternalInput").ap()
    ygT = nc.dram_tensor("ygT", [cfg.D, W], dt.bfloat16, kind="ExternalOutput").ap()

    x_po = cfg.D // P
    dr_po = cfg.DR // P

    with tile.TileContext(nc) as tc, _maybe_loop(tc, loop_n), ExitStack() as ctx:
        const = ctx.enter_context(tc.tile_pool(name="const", bufs=1))
        b1_sb = const.tile([P, cfg.EPC, dr_po], f32)
        nc.sync.dma_start(b1_sb[:], eb1.rearrange("e (po pi) -> pi e po", pi=P))
        if has_b2:
            b2_sb = const.tile([P, cfg.EPC, x_po], f32)
            nc.sync.dma_start(b2_sb[:], eb2.rearrange("e (po pi) -> pi e po", pi=P))
        cwrep = const.tile([P, cfg.EPC, cfg.CAPP], f32)

        hg_shape = ShapeInfo(pdims=((P, dr_po),), fdims=(cfg.CAP,))
        hg_pool = ctx.enter_context(tc.tile_pool(name="hg", bufs=1))
        hg_tiles = [
            [
                hg_pool.tile([P, dr_po, cfg.n_tile], dt.float8e4,
                             tag=f"hg{e}_{n}", name=f"hg{e}_{n}")
                for n in range(cfg.n_tiles)
            ]
            for e in range(cfg.EPC)
        ]

        # layer 1 for all experts first (keeps the PE stream dense)
        for e in range(cfg.EPC):
            with ExitStack() as c2:
                mpool = c2.enter_context(tc.tile_pool(name=f"p2m{e}", bufs=3))
                npool = c2.enter_context(tc.tile_pool(name=f"p2n{e}", bufs=5))
                kxm_prod, kxm_shape = _w_producer(mpool, ew1[e], f"w1_{e}")
                kxn_prod, kxn_shape = dma_from_dram_kxn(
                    npool, xgT[:, ds(e * cfg.CAP, cfg.CAP)]
                )

                def l1_reducer(nc_, psum, sbuf, md, e=e):
                    ko = (md.m_tile_idx * md.m_tile + md.m_subtile_idx * P) // P
                    nc_.scalar.activation(
                        sbuf[:], psum[:], ActFn.Silu,
                        bias=b1_sb[:, e, ko:ko + 1], scale=1.0 / 64.0,
                    )

                def hg_producer(nc_, md, e=e):
                    return hg_tiles[e][md.n_tile_idx][
                        :, ts(md.m_tile_idx, md.m_tile // P), :
                    ]

                composable_matmul_tile_kernel(
                    tc=tc,
                    kxm_shape=kxm_shape,
                    kxn_shape=kxn_shape,
                    output_type=None,
                    kxm_producer=kxm_prod,
                    kxn_producer=kxn_prod,
                    mxn_consumer=lambda nc_, sbuf, md: None,
                    mxn_subtile_reducer=l1_reducer,
                    mxn_subtile_producer=hg_producer,
                    psum_n_bufs=2,
                    MAX_K_TILE_SIZE=1024,
                )

        # cwrep is only needed by layer 2; issue its (broadcast, 2.6MB) DMA
        # after layer 1's tiles so it doesn't delay the first matmuls.
        nc.sync.dma_start(
            cwrep[:],
            cw.rearrange("e c -> (e c)")[None].to_broadcast((P, cfg.EPC * cfg.CAPP)),
        )

        # layer 2 for all experts
        for e in range(cfg.EPC):
            with ExitStack() as c2:
                m2pool = c2.enter_context(tc.tile_pool(name=f"p2m2{e}", bufs=2))
                tpool = c2.enter_context(tc.tile_pool(name=f"p2t{e}", bufs=3))
                kxm2_prod, kxm2_shape = _w_producer(m2pool, ew2[e], f"w2_{e}")

                def hg_kxn_producer(nc_, md, e=e):
                    return hg_tiles[e][md.n_tile_idx][
                        :, ts(md.k_tile_idx, md.k_subtiles), :
                    ]

                def l2_reducer(nc_, psum, sbuf, md, e=e):
                    # no b2: host passes cw/64, so psum*cwrep descales too.
                    # b2: host passes cw; descale psum explicitly, add b2.
                    do = (md.m_tile_idx * md.m_tile + md.m_subtile_idx * P) // P
                    n0 = md.n_tile_idx * md.n_tile + md.n_subtile_idx * md.n_subtile
                    if has_b2:
                        stage = tpool.tile([P, md.n_subtile], dt.float32, tag="stage")
                        nc_.scalar.activation(
                            stage[:], psum[:], ActFn.Copy, scale=1.0 / 64.0
                        )
                        nc_.vector.tensor_scalar_add(
                            stage[:], stage[:], b2_sb[:, e, do:do + 1]
                        )
                        src = stage
                    else:
                        src = psum
                    nc_.vector.tensor_mul(
                        sbuf[:], src[:], cwrep[:, e, ds(n0, md.n_subtile)]
                    )

                composable_matmul_tile_kernel(
                    tc=tc,
                    kxm_shape=kxm2_shape,
                    kxn_shape=hg_shape,
                    output_type=dt.bfloat16,
                    kxm_producer=kxm2_prod,
                    kxn_producer=hg_kxn_producer,
                    mxn_consumer=dma_to_dram_mxn_on(
                        "scalar", ygT[:, ds(e * cfg.CAP, cfg.CAP)]
                    ),
                    mxn_subtile_reducer=l2_reducer,
                    psum_n_bufs=2,
                )

    nc.compile()
    return nc


# --------------------------------------------------------------------------
# Phase 3: combine out = shared + y0 + y1
# --------------------------------------------------------------------------

def build_p3(cfg: Cfg, debug: bool = False, loop_n: int = 0):
    nc = bacc.Bacc("TRN2", target_bir_lowering=False, debug=debug)
    f32 = dt.float32
    aT = nc.dram_tensor("aT", [cfg.D, cfg.TPC], f32, kind="ExternalInput").ap()
    bT = nc.dram_tensor("bT", [cfg.D, cfg.TPC], dt.bfloat16, kind="ExternalInput").ap()
    cT = nc.dram_tensor("cT", [cfg.D, cfg.TPC], dt.bfloat16, kind="ExternalInput").ap()
    oT = nc.dram_tensor("oT", [cfg.D, cfg.TPC], f32, kind="ExternalOutput").ap()

    x_po = cfg.D // P
    CH = 128
    with tile.TileContext(nc) as tc, _maybe_loop(tc, loop_n), ExitStack() as ctx:
        pool = ctx.enter_context(tc.tile_pool(name="sb", bufs=3))
        aT_t, bT_t, cT_t, oT_t = _rearr2(aT), _rearr2(bT), _rearr2(cT), _rearr2(oT)
        for c in range(cfg.TPC // CH):
            a = pool.tile([P, x_po, CH], f32, tag="a")
            nc.sync.dma_start(a[:], aT_t[:, :, ts(c, CH)])
            b = pool.tile([P, x_po, CH], f32, tag="b")
            nc.gpsimd.dma_start(b[:], bT_t[:, :, ts(c, CH)])  # bf16 -> f32 cast
            cc = pool.tile([P, x_po, CH], f32, tag="c")
            nc.gpsimd.dma_start(cc[:], cT_t[:, :, ts(c, CH)])
            nc.vector.tensor_add(a[:], a[:], b[:])
            nc.vector.tensor_add(a[:], a[:], cc[:])
            nc.sync.dma_start(oT_t[:, :, ts(c, CH)], a[:])
    nc.compile()
    return nc


# --------------------------------------------------------------------------
# Host orchestration
# --------------------------------------------------------------------------

def _get(phase: str, cfg: Cfg, **bkw):
    key = (phase, cfg, tuple(sorted(bkw.items())))
    if key not in _cache:
        _cache[key] = {
            "p1": build_p1, "p2": build_p2, "p3": build_p3,
            "p1r": build_p1r, "p3s": build_p3s,
        }[phase](cfg, **bkw)
    return _cache[key]


CAPTURE: dict | None = None


def _run(phase: str, cfg: Cfg, in_maps, **bkw):
    if CAPTURE is not None:
        CAPTURE[phase] = (cfg, in_maps, bkw)
    nc = _get(phase, cfg, **bkw)
    r = run_bass_kernel_spmd(nc, in_maps, core_ids=list(range(cfg.n_cores)), trace=TRACE)
    LAST_EXEC_NS[phase] = r.exec_time_ns
    return r.results


def kernel(**inputs) -> np.ndarray:
    cfg = CFG
    x = np.ascontiguousarray(np.asarray(inputs["x"], dtype=np.float32))
    Bn, S, D = x.shape
    assert (Bn, S, D) == (cfg.n_cores, cfg.TPC, cfg.D)
    step_t = int(np.asarray(inputs["step_t"]))
    rw = np.ascontiguousarray(np.asarray(inputs["router_w"], np.float32)[step_t])
    re_w1 = np.ascontiguousarray(np.asarray(inputs["re_w1"], np.float32))
    re_b1 = np.ascontiguousarray(np.asarray(inputs["re_b1"], np.float32))
    re_w2 = np.ascontiguousarray(np.asarray(inputs["re_w2"], np.float32))
    re_b2 = np.ascontiguousarray(np.asarray(inputs["re_b2"], np.float32))
    sh_w1 = np.ascontiguousarray(np.asarray(inputs["sh_w1"], np.float32))
    sh_b1 = np.ascontiguousarray(np.asarray(inputs["sh_b1"], np.float32))
    sh_w2 = np.ascontiguousarray(np.asarray(inputs["sh_w2"], np.float32))
    sh_b2 = np.ascontiguousarray(np.asarray(inputs["sh_b2"], np.float32))

    xT = np.ascontiguousarray(x.transpose(0, 2, 1))  # [B, D, S] feature-major

    # ---- phase 1: router ----
    ident = np.eye(cfg.E, dtype=np.float32)
    in1 = [{"xT": xT[b], "rw": rw, "ident": ident} for b in range(cfg.n_cores)]
    r1 = _run("p1r", cfg, in1)

    idx = np.stack([r["ridx"][:, :2] for r in r1]).astype(np.int64)   # [B, S, 2]
    wts = np.stack([r["rwts"][:, :2] for r in r1])                    # [B, S, 2] f32

    T = Bn * S
    pair_e = idx.reshape(-1)                   # expert of pair p (p = g*2 + k)
    order = np.argsort(pair_e, kind="stable")  # pairs sorted by expert
    counts = np.bincount(pair_e, minlength=cfg.E)

    if counts.max() > cfg.CAP:  # safety net: regrow capacity, rebuild p2
        cfg = Cfg(CAP=int(-(-(counts.max() + 64) // P) * P))

    seg = np.zeros(cfg.E + 1, np.int64)
    seg[1:] = np.cumsum(counts)
    cols = np.zeros((cfg.E, cfg.CAP), np.int64)           # token col in xball
    cwarr = np.zeros((cfg.E, cfg.CAPP), np.float32)       # combine weights
    pos_of_pair = np.empty(2 * T, np.int64)
    wflat = wts.reshape(-1)
    for e in range(cfg.E):
        sl = order[seg[e]:seg[e + 1]]
        n = len(sl)
        cols[e, :n] = sl // 2
        cwarr[e, :n] = wflat[sl]
        pos_of_pair[sl] = np.arange(n)

    # ---- phase 2 (fp8: x/h at scale 1, weights at scale 64) ----
    has_b2 = bool(np.any(re_b2))
    F8 = ml_dtypes.float8_e4m3
    xf8 = np.ascontiguousarray(
        xT.transpose(1, 0, 2).reshape(cfg.D, -1)
    ).astype(F8)                                           # [D, T] fp8
    w1f8 = np.clip(re_w1 * 64.0, -240, 240).astype(F8)
    w2f8 = np.clip(re_w2 * 64.0, -240, 240).astype(F8)
    cw_pass = cwarr if has_b2 else cwarr * (1.0 / 64.0)
    in2 = []
    for c in range(cfg.n_cores):
        e0 = c * cfg.EPC
        xg = xf8[:, cols[e0:e0 + cfg.EPC].reshape(-1)]    # [D, EPC*CAP] fp8
        in2.append({
            "xgT": np.ascontiguousarray(xg),
            "ew1": np.ascontiguousarray(w1f8[e0:e0 + cfg.EPC]),
            "eb1": re_b1[e0:e0 + cfg.EPC],
            "ew2": np.ascontiguousarray(w2f8[e0:e0 + cfg.EPC]),
            "eb2": re_b2[e0:e0 + cfg.EPC],
            "cw": cw_pass[e0:e0 + cfg.EPC],
        })
    r2 = _run("p2", cfg, in2, has_b2=has_b2)

    # global y layout: expert e occupies columns [e*CAP, (e+1)*CAP)
    yall = np.concatenate([r["ygT"] for r in r2], axis=1)  # [D, E*CAP] bf16

    ycol_of_pair = pair_e * cfg.CAP + pos_of_pair          # [2T]
    ya = yall[:, ycol_of_pair[0::2]]                       # [D, T] slot k=0
    yb = yall[:, ycol_of_pair[1::2]]                       # [D, T] slot k=1

    # ---- phase 3: shared experts + combine ----
    sh_w1bf = np.ascontiguousarray(
        np.concatenate([sh_w1[0], sh_w1[1]], axis=1).astype(ml_dtypes.bfloat16)
    )
    sh_w2bf = sh_w2.astype(ml_dtypes.bfloat16)
    in3 = [
        {
            "xT": xT[b],
            "sw1": sh_w1bf, "sb1": sh_b1, "sw2": sh_w2bf, "sb2": sh_b2,
            "yaT": np.ascontiguousarray(ya[:, b * S:(b + 1) * S]),
            "ybT": np.ascontiguousarray(yb[:, b * S:(b + 1) * S]),
        }
        for b in range(cfg.n_cores)
    ]
    r3 = _run("p3s", cfg, in3, has_b2=bool(np.any(sh_b2)))

    out = np.stack([r["outT"] for r in r3])                # [B, D, S]
    return np.ascontiguousarray(out.transpose(0, 2, 1))    # [B, S, D] f32



# revision 34
# speedup vs baseline: 1.0329x; 1.0329x over previous
"""Trainium2 Bass kernel for nn_ChainOfExperts (MoE with shared experts).

Strategy (8 NeuronCores):
  Phase 1 (data-parallel, tokens sharded along B): router logits + top-2
    softmax weights on-device, shared-expert FFN, and a bf16 cast of x.
  Host: pure data movement — group token slots by routed expert (counting
    sort on device-computed indices), gather bf16 token columns per expert.
  Phase 2 (expert-parallel, 2 experts per core): routed-expert FFN on the
    gathered tokens (capacity-padded), output pre-scaled by combine weight.
  Phase 3 (data-parallel): out = shared + y_slot0 + y_slot1.

All activations are kept feature-major ([D, tokens]) so every matmul has its
contraction dim on partitions. Matmuls run in bf16 (fp32 accumulate); the
router runs in fp32.
"""

import os
from contextlib import ExitStack
from dataclasses import dataclass

import numpy as np
import ml_dtypes

import concourse.bass as bass
import concourse.mybir as mybir
import concourse.tile as tile
from concourse import bacc
from concourse.bass import ts, ds, broadcast_tensor_aps
from concourse.bass_utils import run_bass_kernel_spmd
from concourse.kernels.tile_matmul import (
    ShapeInfo,
    composable_matmul_tile_kernel,
    cast_to_type,
    dma_from_dram_kxm,
    dma_from_dram_kxn,
    dma_to_dram_mxn,
    _tiled_ap,
)


def dma_to_dram_mxn_on(engine_name: str, ap):
    """dma_to_dram_mxn variant issuing on a chosen engine, spreading DMA
    descriptor generation off the (busy) sync sequencer."""
    ap, shape = _tiled_ap(ap)

    def consume(nc_, mxn_tile, md):
        n_sz = min(md.n_tile, shape.fdims[0] - md.n_tile_idx * md.n_tile)
        getattr(nc_, engine_name).dma_start(
            ap[
                :, ts(md.m_tile_idx, md.m_subtiles),
                ds(md.n_tile_idx * md.n_tile, n_sz),
            ],
            mxn_tile[:, :, :n_sz],
        )

    return consume

BF16 = ml_dtypes.bfloat16
ActFn = mybir.ActivationFunctionType
dt = mybir.dt
P = 128

# bass_utils imports antenv.axon_hooks when tracing is requested; this
# container ships only an antenv stub. Provide the missing module, wired
# to the axon ctypes NTFF hook when the injected .so supports it, so a
# trace request yields real device profiles instead of crashing.
import sys as _sys
try:
    import antenv.axon_hooks  # noqa: F401
except ImportError:
    import types as _types
    import antenv as _antenv
    _stub = _types.ModuleType("antenv.axon_hooks")
    _hook_box = [None]
    _stub.get_axon_ntff_profile_hook = lambda: _hook_box[0]
    _stub.set_axon_ntff_profile_hook = lambda h: _hook_box.__setitem__(0, h)
    _sys.modules["antenv.axon_hooks"] = _stub
    _antenv.axon_hooks = _stub
    try:
        from trn_agent_boot.trn_boot import _ntff_profile_via_ctypes
        _hook_box[0] = _ntff_profile_via_ctypes("/opt/axon/libaxon_pjrt.so")
    except Exception:
        pass


@dataclass(frozen=True)
class Cfg:
    n_cores: int = 8
    D: int = 2048     # hidden dim
    TPC: int = 2048   # tokens per core
    E: int = 16       # routed experts
    NSH: int = 2      # shared experts
    DS: int = 1024    # shared inner dim
    DR: int = 512     # routed inner dim
    CAP: int = 2304   # per-expert token capacity (multiple of 128)
    EPC: int = 2      # experts per core

    @property
    def n_tile(self):  # composable's N tiling for N=CAP
        return min(512, -(-self.CAP // P) * P)

    @property
    def n_tiles(self):
        return -(-self.CAP // self.n_tile)

    @property
    def CAPP(self):  # hg cache width: CAP padded to whole n-tiles
        return self.n_tiles * self.n_tile


CFG = Cfg()
TRACE = bool(os.environ.get("KERNEL_TRACE"))
LAST_EXEC_NS: dict[str, int | None] = {}

_cache: dict = {}


def _rearr2(ap):
    """[K, N] dram AP -> [pi, po, N] with K = po*128 + pi."""
    return ap.rearrange("(po pi) t -> pi po t", pi=P)


# --------------------------------------------------------------------------
# Phase 1: router + shared experts + bf16 cast of x
# --------------------------------------------------------------------------

class _NullCtx:
    def __enter__(self):
        return None

    def __exit__(self, *a):
        return False


def _maybe_loop(tc, loop_n):
    """Wrap the phase body in an in-NEFF repeat loop (for benchmarking)."""
    return tc.For_i(0, loop_n, 1) if loop_n else _NullCtx()


def build_p1(cfg: Cfg, debug: bool = False, loop_n: int = 0):
    nc = bacc.Bacc("TRN2", target_bir_lowering=False, debug=debug)
    f32 = dt.float32
    xT = nc.dram_tensor("xT", [cfg.D, cfg.TPC], f32, kind="ExternalInput").ap()
    rw = nc.dram_tensor("rw", [cfg.D, cfg.E], f32, kind="ExternalInput").ap()
    sw1 = nc.dram_tensor("sw1", [cfg.NSH, cfg.D, cfg.DS], f32, kind="ExternalInput").ap()
    sb1 = nc.dram_tensor("sb1", [cfg.NSH, cfg.DS], f32, kind="ExternalInput").ap()
    sw2 = nc.dram_tensor("sw2", [cfg.NSH, cfg.DS, cfg.D], f32, kind="ExternalInput").ap()
    sb2 = nc.dram_tensor("sb2", [cfg.NSH, cfg.D], f32, kind="ExternalInput").ap()
    out_shT = nc.dram_tensor("out_shT", [cfg.D, cfg.TPC], f32, kind="ExternalOutput").ap()
    xbfT = nc.dram_tensor("xbfT", [cfg.D, cfg.TPC], dt.bfloat16, kind="ExternalOutput").ap()
    ridx = nc.dram_tensor("ridx", [cfg.TPC, 8], dt.uint32, kind="ExternalOutput").ap()
    rwts = nc.dram_tensor("rwts", [cfg.TPC, 8], f32, kind="ExternalOutput").ap()
    h_dram = nc.dram_tensor("h_mid", [cfg.NSH, cfg.DS, cfg.TPC], dt.bfloat16).ap()

    x_po = cfg.D // P
    ds_po = cfg.DS // P
    CH = 256  # router/cast chunk (tokens)

    with tile.TileContext(nc) as tc, _maybe_loop(tc, loop_n), ExitStack() as ctx:
        const = ctx.enter_context(tc.tile_pool(name="const", bufs=1))
        rw_sb = const.tile([P, x_po, cfg.E], f32)
        nc.sync.dma_start(rw_sb[:], rw.rearrange("(po pi) e -> pi po e", pi=P))
        b1_sb = const.tile([P, cfg.NSH, ds_po], f32)
        nc.sync.dma_start(b1_sb[:], sb1.rearrange("s (po pi) -> pi s po", pi=P))
        b2_sb = const.tile([P, cfg.NSH, x_po], f32)
        nc.sync.dma_start(b2_sb[:], sb2.rearrange("s (po pi) -> pi s po", pi=P))
        b2sum = const.tile([P, x_po], f32)
        nc.vector.tensor_add(b2sum[:], b2_sb[:, 0], b2_sb[:, 1])
        xbf_cache = const.tile([P, x_po, cfg.TPC], dt.bfloat16)

        # ---- router + cast pass ----
        with ExitStack() as c2:
            xsrc = c2.enter_context(tc.tile_pool(name="xsrc", bufs=2))
            rps = c2.enter_context(tc.tile_pool(name="rpsum", bufs=2, space="PSUM"))
            rsb = c2.enter_context(tc.tile_pool(name="rsb", bufs=3))
            xT_t = _rearr2(xT)
            xbfT_t = _rearr2(xbfT)
            for c in range(cfg.TPC // CH):
                xt = xsrc.tile([P, x_po, CH], f32, tag="xt")
                nc.sync.dma_start(xt[:], xT_t[:, :, ts(c, CH)])
                nc.vector.tensor_copy(xbf_cache[:, :, ts(c, CH)], xt[:])
                nc.sync.dma_start(xbfT_t[:, :, ts(c, CH)], xbf_cache[:, :, ts(c, CH)])
                for tt in range(CH // P):
                    t0 = c * CH + tt * P
                    ps = rps.tile([P, cfg.E], f32, tag="rp")
                    for po in range(x_po):
                        nc.tensor.matmul(
                            ps[:], xt[:, po, ts(tt, P)], rw_sb[:, po, :],
                            start=(po == 0), stop=(po == x_po - 1),
                        )
                    lg = rsb.tile([P, cfg.E], f32, tag="lg")
                    nc.vector.tensor_copy(lg[:], ps[:])
                    mx = rsb.tile([P, 8], f32, tag="mx")
                    nc.vector.max(mx[:], lg[:])
                    ix = rsb.tile([P, 8], dt.uint32, tag="ix")
                    nc.vector.max_index(ix[:], mx[:], lg[:])
                    nm = rsb.tile([P, 1], f32, tag="nm")
                    nc.vector.tensor_scalar_mul(nm[:], mx[:, 0:1], -1.0)
                    ex = rsb.tile([P, cfg.E], f32, tag="ex")
                    zz = rsb.tile([P, 1], f32, tag="zz")
                    nc.scalar.activation(ex[:], lg[:], ActFn.Exp, bias=nm[:], accum_out=zz[:])
                    rz = rsb.tile([P, 1], f32, tag="rz")
                    nc.vector.reciprocal(rz[:], zz[:])
                    wv = rsb.tile([P, 8], f32, tag="wv")
                    nc.scalar.activation(wv[:], mx[:], ActFn.Exp, bias=nm[:])
                    nc.vector.tensor_scalar_mul(wv[:], wv[:], rz[:])
                    nc.sync.dma_start(ridx[ds(t0, P), :], ix[:])
                    nc.sync.dma_start(rwts[ds(t0, P), :], wv[:])

        # ---- shared experts layer 1 (per shared expert s) ----
        xbf_shape = ShapeInfo(pdims=((P, x_po),), fdims=(cfg.TPC,))

        def xbf_producer(nc_, md):
            return xbf_cache[
                :, ts(md.k_tile_idx, md.k_subtiles),
                ds(md.n_tile_idx * md.n_tile, md.n_tile)
            ]

        for s in range(cfg.NSH):
            with ExitStack() as c2:
                mpool = c2.enter_context(tc.tile_pool(name=f"l1m{s}", bufs=2))
                cpool = c2.enter_context(tc.tile_pool(name=f"l1c{s}", bufs=5))
                kxm_prod, kxm_shape = dma_from_dram_kxm(mpool, sw1[s])
                kxm_prod = cast_to_type(kxm_prod, cpool, dt.bfloat16)

                def l1_reducer(nc_, psum, sbuf, md, s=s):
                    ko = (md.m_tile_idx * md.m_tile + md.m_subtile_idx * P) // P
                    nc_.scalar.activation(
                        sbuf[:], psum[:], ActFn.Silu, bias=b1_sb[:, s, ko:ko + 1]
                    )

                composable_matmul_tile_kernel(
                    tc=tc,
                    kxm_shape=kxm_shape,
                    kxn_shape=xbf_shape,
                    output_type=dt.bfloat16,
                    kxm_producer=kxm_prod,
                    kxn_producer=xbf_producer,
                    mxn_consumer=dma_to_dram_mxn(h_dram[s]),
                    mxn_subtile_reducer=l1_reducer,
                )

        # ---- shared experts layer 2 (contract over s and DS jointly) ----
        with ExitStack() as c2:
            mpool = c2.enter_context(tc.tile_pool(name="l2m", bufs=2))
            cpool = c2.enter_context(tc.tile_pool(name="l2c", bufs=5))
            npool = c2.enter_context(tc.tile_pool(name="l2n", bufs=5))
            kxm_prod, kxm_shape = dma_from_dram_kxm(mpool, sw2, batch_k=True)
            kxm_prod = cast_to_type(kxm_prod, cpool, dt.bfloat16)
            kxn_prod, kxn_shape = dma_from_dram_kxn(npool, h_dram, batch_k=True)

            def l2_reducer(nc_, psum, sbuf, md):
                do = (md.m_tile_idx * md.m_tile + md.m_subtile_idx * P) // P
                nc_.vector.tensor_scalar_add(sbuf[:], psum[:], b2sum[:, do:do + 1])

            composable_matmul_tile_kernel(
                tc=tc,
                kxm_shape=kxm_shape,
                kxn_shape=kxn_shape,
                output_type=dt.float32,
                kxm_producer=kxm_prod,
                kxn_producer=kxn_prod,
                mxn_consumer=dma_to_dram_mxn(out_shT),
                mxn_subtile_reducer=l2_reducer,
            )

    nc.compile()
    return nc


def build_p1r(cfg: Cfg, debug: bool = False, loop_n: int = 0):
    """Router-only phase: top-2 indices/weights.

    Router matmuls keep rw stationary (16-col LDWEIGHTS, ~13ns) with x as
    the moving operand, accumulating transposed logits [E, TPC] in PSUM
    across all D subtiles. PE transposes then yield [128-token, E] groups
    for a top-2 softmax batched across all groups (one DVE op per step).
    """
    nc = bacc.Bacc("TRN2", target_bir_lowering=False, debug=debug)
    f32 = dt.float32
    E = cfg.E
    xT = nc.dram_tensor("xT", [cfg.D, cfg.TPC], f32, kind="ExternalInput").ap()
    rw = nc.dram_tensor("rw", [cfg.D, E], f32, kind="ExternalInput").ap()
    ident = nc.dram_tensor("ident", [E, E], f32, kind="ExternalInput").ap()
    ridx = nc.dram_tensor("ridx", [cfg.TPC, 8], dt.uint32, kind="ExternalOutput").ap()
    rwts = nc.dram_tensor("rwts", [cfg.TPC, 8], f32, kind="ExternalOutput").ap()

    x_po = cfg.D // P
    PO_CH = 1
    NPO = x_po // PO_CH
    NG = cfg.TPC // P
    NT = cfg.TPC // 512

    with tile.TileContext(nc) as tc, _maybe_loop(tc, loop_n), ExitStack() as ctx:
        const = ctx.enter_context(tc.tile_pool(name="const", bufs=1))
        rw_sb = const.tile([P, x_po, E], f32)
        nc.sync.dma_start(rw_sb[:], rw.rearrange("(po pi) e -> pi po e", pi=P))
        id_sb = const.tile([E, E], f32)
        nc.sync.dma_start(id_sb[:], ident)

        xsrc = ctx.enter_context(tc.tile_pool(name="xsrc", bufs=6))
        lgps = ctx.enter_context(tc.tile_pool(name="lgps", bufs=1, space="PSUM"))
        tps = ctx.enter_context(tc.tile_pool(name="tps", bufs=1, space="PSUM"))
        sm = ctx.enter_context(tc.tile_pool(name="sm", bufs=1))
        xT_t = _rearr2(xT)

        lgT_ps = lgps.tile([E, cfg.TPC], f32)
        for po8 in range(NPO):
            xt = xsrc.tile([P, PO_CH, cfg.TPC], f32, tag="xt")
            nc.sync.dma_start(xt[:], xT_t[:, ts(po8, PO_CH), :])
            for pp in range(PO_CH):
                po = po8 * PO_CH + pp
                for t in range(NT):
                    nc.tensor.matmul(
                        lgT_ps[:, ts(t, 512)], rw_sb[:, po, :],
                        xt[:, pp, ts(t, 512)],
                        start=(po == 0), stop=(po == x_po - 1),
                        skip_group_check=True,
                    )

        lgT_sb = sm.tile([E, cfg.TPC], f32, tag="lgT")
        nc.vector.tensor_copy(lgT_sb[:], lgT_ps[:])
        lg_ps = tps.tile([P, NG, E], f32)
        for g in range(NG):
            nc.tensor.transpose(lg_ps[:, g, :], lgT_sb[:, ts(g, P)], id_sb[:])
        lg_all = sm.tile([P, NG, E], f32, tag="lg")
        nc.vector.tensor_copy(lg_all[:], lg_ps[:])

        mx = sm.tile([P, NG, 8], f32, tag="mx")
        ix = sm.tile([P, NG, 8], dt.uint32, tag="ix")
        for g in range(NG):
            nc.vector.max(mx[:, g, :], lg_all[:, g, :])
            nc.vector.max_index(ix[:, g, :], mx[:, g, :], lg_all[:, g, :])

        # batched stable softmax: nm = -top1; w_k = exp(m_k+nm)/sum(exp(lg+nm))
        nm = sm.tile([P, NG, 1], f32, tag="nm")
        nc.vector.tensor_scalar_mul(nm[:], mx[:, :, 0:1], -1.0)
        d8 = sm.tile([P, NG, 8], f32, tag="d8")
        a, b = broadcast_tensor_aps(mx[:], nm[:])
        nc.vector.tensor_add(d8[:], a, b)
        wv = sm.tile([P, NG, 8], f32, tag="wv")
        nc.scalar.activation(wv[:], d8[:], ActFn.Exp)
        dE = sm.tile([P, NG, E], f32, tag="dE")
        a, b = broadcast_tensor_aps(lg_all[:], nm[:])
        nc.vector.tensor_add(dE[:], a, b)
        ex = sm.tile([P, NG, E], f32, tag="ex")
        nc.scalar.activation(ex[:], dE[:], ActFn.Exp)
        zz = sm.tile([P, NG], f32, tag="zz")
        nc.vector.tensor_reduce(
            zz[:], ex[:], mybir.AxisListType.X, mybir.AluOpType.add
        )
        rz = sm.tile([P, NG, 1], f32, tag="rz")
        nc.vector.reciprocal(rz[:], zz[:])
        a, b = broadcast_tensor_aps(wv[:], rz[:])
        nc.vector.tensor_mul(wv[:], a, b)

        nc.sync.dma_start(ridx.rearrange("(g pi) k -> pi g k", pi=P), ix[:])
        nc.sync.dma_start(rwts.rearrange("(g pi) k -> pi g k", pi=P), wv[:])
    nc.compile()
    return nc


def _w_producer_batched(pool, w_ap, tagname):
    """Batched-K variant of _w_producer for [S, K, M] weights."""
    S, K, M = w_ap.shape
    shape = ShapeInfo(pdims=((P, K // P),) * S, fdims=(M,))
    w_ts = [w_ap[s].rearrange("(po pi) m -> pi po m", pi=P) for s in range(S)]
    cast = w_ap.dtype == dt.float32

    def prod(nc_, md):
        t = pool.tile([P, md.k_subtiles, md.m_tile],
                      dt.bfloat16 if cast else w_ap.dtype, tag=tagname)
        dma = nc_.gpsimd.dma_start if cast else nc_.sync.dma_start
        dma(
            t[:],
            w_ts[md.k_batch_idx][
                :, ts(md.k_tile_idx, md.k_subtiles),
                ds(md.m_tile_idx * md.m_tile, md.m_tile)
            ],
        )
        return t

    return prod, shape


def build_p3s(cfg: Cfg, debug: bool = False, loop_n: int = 0, has_b2: bool = False):
    """Shared experts + combine: out = sharedFFN(x) + ya + yb.

    h is kept in SBUF as per-(s, n-tile) tiles so layer 2 pipelines with
    layer 1 at tile granularity (no DRAM roundtrip, no coarse-dep stall).
    """
    nc = bacc.Bacc("TRN2", target_bir_lowering=False, debug=debug)
    f32 = dt.float32
    xT = nc.dram_tensor("xT", [cfg.D, cfg.TPC], f32, kind="ExternalInput").ap()
    # sw1 is the two shared-expert w1s concatenated on the inner dim (host)
    sw1 = nc.dram_tensor("sw1", [cfg.D, cfg.NSH * cfg.DS], dt.bfloat16, kind="ExternalInput").ap()
    sb1 = nc.dram_tensor("sb1", [cfg.NSH, cfg.DS], f32, kind="ExternalInput").ap()
    sw2 = nc.dram_tensor("sw2", [cfg.NSH, cfg.DS, cfg.D], dt.bfloat16, kind="ExternalInput").ap()
    sb2 = nc.dram_tensor("sb2", [cfg.NSH, cfg.D], f32, kind="ExternalInput").ap()
    yaT = nc.dram_tensor("yaT", [cfg.D, cfg.TPC], dt.bfloat16, kind="ExternalInput").ap()
    ybT = nc.dram_tensor("ybT", [cfg.D, cfg.TPC], dt.bfloat16, kind="ExternalInput").ap()
    outT = nc.dram_tensor("outT", [cfg.D, cfg.TPC], f32, kind="ExternalOutput").ap()

    x_po = cfg.D // P
    ds_po = cfg.DS // P
    NT = cfg.TPC // 512  # n tiles

    with tile.TileContext(nc) as tc, _maybe_loop(tc, loop_n), ExitStack() as ctx:
        const = ctx.enter_context(tc.tile_pool(name="const", bufs=1))
        b1_sb = const.tile([P, cfg.NSH, ds_po], f32)
        nc.sync.dma_start(b1_sb[:], sb1.rearrange("s (po pi) -> pi s po", pi=P))
        b2_sb = const.tile([P, cfg.NSH, x_po], f32)
        nc.sync.dma_start(b2_sb[:], sb2.rearrange("s (po pi) -> pi s po", pi=P))
        b2sum = const.tile([P, x_po], f32)
        nc.vector.tensor_add(b2sum[:], b2_sb[:, 0], b2_sb[:, 1])

        hpool = ctx.enter_context(tc.tile_pool(name="hp", bufs=1))
        h_tiles = [
            [
                hpool.tile([P, ds_po, 512], dt.bfloat16,
                           tag=f"h{s}_{n}", name=f"h{s}_{n}")
                for n in range(NT)
            ]
            for s in range(cfg.NSH)
        ]

        with ExitStack() as c2:
            mpool = c2.enter_context(tc.tile_pool(name="l1m", bufs=6))
            npool = c2.enter_context(tc.tile_pool(name="l1n", bufs=3))
            cpool = c2.enter_context(tc.tile_pool(name="l1c", bufs=6))
            kxm_prod, kxm_shape = _w_producer(mpool, sw1, "sw1")
            kxn_prod, kxn_shape = dma_from_dram_kxn(npool, xT)
            kxn_prod = cast_to_type(kxn_prod, cpool, dt.bfloat16)

            def l1_reducer(nc_, psum, sbuf, md):
                ko_abs = (md.m_tile_idx * md.m_tile + md.m_subtile_idx * P) // P
                s, ko = divmod(ko_abs, ds_po)
                nc_.scalar.activation(
                    sbuf[:], psum[:], ActFn.Silu, bias=b1_sb[:, s, ko:ko + 1]
                )

            def h_producer(nc_, md):
                ko_abs = (md.m_tile_idx * md.m_tile) // P
                s, off = divmod(ko_abs, ds_po)
                return h_tiles[s][md.n_tile_idx][
                    :, ds(off, md.m_tile // P), :
                ]

            composable_matmul_tile_kernel(
                tc=tc,
                kxm_shape=kxm_shape,
                kxn_shape=kxn_shape,
                output_type=None,
                kxm_producer=kxm_prod,
                kxn_producer=kxn_prod,
                mxn_consumer=lambda nc_, sbuf, md: None,
                mxn_subtile_reducer=l1_reducer,
                mxn_subtile_producer=h_producer,
                psum_n_bufs=2,
                MAX_K_TILE_SIZE=512,
            )

        with ExitStack() as c2:
            mpool = c2.enter_context(tc.tile_pool(name="l2m", bufs=4))
            apool = c2.enter_context(tc.tile_pool(name="addp", bufs=3))
            kxm_prod, kxm_shape = _w_producer_batched(mpool, sw2, "sw2")
            kxn_shape = ShapeInfo(pdims=((P, ds_po),) * cfg.NSH, fdims=(cfg.TPC,))

            def h_kxn_producer(nc_, md):
                return h_tiles[md.k_batch_idx][md.n_tile_idx][
                    :, ts(md.k_tile_idx, md.k_subtiles), :
                ]

            def l2_reducer(nc_, psum, sbuf, md):
                do = md.m_tile_idx * (md.m_tile // P) + md.m_subtile_idx
                if has_b2:
                    nc_.vector.tensor_scalar_add(sbuf[:], psum[:], b2sum[:, do:do + 1])
                else:
                    nc_.vector.tensor_copy(sbuf[:], psum[:])

            base_consumer = dma_to_dram_mxn(outT)
            yaT_t, ybT_t = _rearr2(yaT), _rearr2(ybT)

            def combine_consumer(nc_, sbuf, md):
                po0 = md.m_tile_idx * (md.m_tile // P)
                nsub = md.m_tile // P
                nsl = ds(md.n_tile_idx * md.n_tile, md.n_tile)
                ya_t = apool.tile([P, nsub, md.n_tile], dt.bfloat16, tag="ya")
                nc_.sync.dma_start(ya_t[:], yaT_t[:, ds(po0, nsub), nsl])
                yb_t = apool.tile([P, nsub, md.n_tile], dt.bfloat16, tag="yb")
                nc_.sync.dma_start(yb_t[:], ybT_t[:, ds(po0, nsub), nsl])
                nc_.vector.tensor_add(sbuf[:], sbuf[:], ya_t[:])
                nc_.vector.tensor_add(sbuf[:], sbuf[:], yb_t[:])
                base_consumer(nc_, sbuf, md)

            composable_matmul_tile_kernel(
                tc=tc,
                kxm_shape=kxm_shape,
                kxn_shape=kxn_shape,
                output_type=dt.float32,
                kxm_producer=kxm_prod,
                kxn_producer=h_kxn_producer,
                mxn_consumer=combine_consumer,
                mxn_subtile_reducer=l2_reducer,
                psum_n_bufs=2,
                MAX_K_TILE_SIZE=1024,
            )
    nc.compile()
    return nc


# --------------------------------------------------------------------------
# Phase 2: routed experts (expert-parallel, capacity padded)
# --------------------------------------------------------------------------

def _w_producer(pool, w_ap, tagname):
    """kxm producer streaming a [K, M] weight from DRAM. f32 sources are
    cast to bf16 via SWDGE cast-DMA; bf16/fp8 sources stream as-is."""
    K, M = w_ap.shape
    shape = ShapeInfo(pdims=((P, K // P),), fdims=(M,))
    w_t = w_ap.rearrange("(po pi) m -> pi po m", pi=P)
    cast = w_ap.dtype == dt.float32

    def prod(nc_, md):
        t = pool.tile(
            [P, md.k_subtiles, md.m_tile],
            dt.bfloat16 if cast else w_ap.dtype, tag=tagname
        )
        dma = nc_.gpsimd.dma_start if cast else nc_.sync.dma_start
        dma(
            t[:],
            w_t[
                :, ts(md.k_tile_idx, md.k_subtiles),
                ds(md.m_tile_idx * md.m_tile, md.m_tile)
            ],
        )
        return t

    return prod, shape


def build_p2(cfg: Cfg, debug: bool = False, loop_n: int = 0, has_b2: bool = False):
    """Routed-expert FFN in fp8 (DoubleRow matmuls).

    Host pre-scales w1/w2 by 64 and casts to fp8e4; the 1/64 descale is
    folded into the silu activation (layer 1) and the combine weights
    (layer 2). x and h are fp8 at scale 1.
    """
    nc = bacc.Bacc("TRN2", target_bir_lowering=False, debug=debug)
    f32 = dt.float32
    f8 = dt.float8e4
    W = cfg.EPC * cfg.CAP
    xgT = nc.dram_tensor("xgT", [cfg.D, W], f8, kind="ExternalInput").ap()
    ew1 = nc.dram_tensor("ew1", [cfg.EPC, cfg.D, cfg.DR], f8, kind="ExternalInput").ap()
    eb1 = nc.dram_tensor("eb1", [cfg.EPC, cfg.DR], f32, kind="ExternalInput").ap()
    ew2 = nc.dram_tensor("ew2", [cfg.EPC, cfg.DR, cfg.D], f8, kind="ExternalInput").ap()
    eb2 = nc.dram_tensor("eb2", [cfg.EPC, cfg.D], f32, kind="ExternalInput").ap()
    cw = nc.dram_tensor("cw", [cfg.EPC, cfg.CAPP], f32, kind="ExternalInput").ap()
    ygT = nc.dram_tensor("ygT", [cfg.D, W], dt.bfloat16, kind="ExternalOutput").ap()

    x_po = cfg.D // P
    dr_po = cfg.DR // P

    with tile.TileContext(nc) as tc, _maybe_loop(tc, loop_n), ExitStack() as ctx:
        const = ctx.enter_context(tc.tile_pool(name="const", bufs=1))
        b1_sb = const.tile([P, cfg.EPC, dr_po], f32)
        nc.sync.dma_start(b1_sb[:], eb1.rearrange("e (po pi) -> pi e po", pi=P))
        if has_b2:
            b2_sb = const.tile([P, cfg.EPC, x_po], f32)
            nc.sync.dma_start(b2_sb[:], eb2.rearrange("e (po pi) -> pi e po", pi=P))
        cwrep = const.tile([P, cfg.EPC, cfg.CAPP], f32)

        hg_shape = ShapeInfo(pdims=((P, dr_po),), fdims=(cfg.CAP,))
        hg_pool = ctx.enter_context(tc.tile_pool(name="hg", bufs=1))
        hg_tiles = [
            [
                hg_pool.tile([P, dr_po, cfg.n_tile], dt.float8e4,
                             tag=f"hg{e}_{n}", name=f"hg{e}_{n}")
                for n in range(cfg.n_tiles)
            ]
            for e in range(cfg.EPC)
        ]

        # layer 1 for all experts first (keeps the PE stream dense)
        for e in range(cfg.EPC):
            with ExitStack() as c2:
                mpool = c2.enter_context(tc.tile_pool(name=f"p2m{e}", bufs=6))
                npool = c2.enter_context(tc.tile_pool(name=f"p2n{e}", bufs=6))
                kxm_prod, kxm_shape = _w_producer(mpool, ew1[e], f"w1_{e}")
                kxn_prod, kxn_shape = dma_from_dram_kxn(
                    npool, xgT[:, ds(e * cfg.CAP, cfg.CAP)]
                )

                def l1_reducer(nc_, psum, sbuf, md, e=e):
                    ko = (md.m_tile_idx * md.m_tile + md.m_subtile_idx * P) // P
                    nc_.scalar.activation(
                        sbuf[:], psum[:], ActFn.Silu,
                        bias=b1_sb[:, e, ko:ko + 1], scale=1.0 / 64.0,
                    )

                def hg_producer(nc_, md, e=e):
                    return hg_tiles[e][md.n_tile_idx][
                        :, ts(md.m_tile_idx, md.m_tile // P), :
                    ]

                composable_matmul_tile_kernel(
                    tc=tc,
                    kxm_shape=kxm_shape,
                    kxn_shape=kxn_shape,
                    output_type=None,
                    kxm_producer=kxm_prod,
                    kxn_producer=kxn_prod,
                    mxn_consumer=lambda nc_, sbuf, md: None,
                    mxn_subtile_reducer=l1_reducer,
                    mxn_subtile_producer=hg_producer,
                    psum_n_bufs=2,
                    MAX_K_TILE_SIZE=512,
                )

        # cwrep is only needed by layer 2; issue its (broadcast, 2.6MB) DMA
        # after layer 1's tiles so it doesn't delay the first matmuls.
        nc.sync.dma_start(
            cwrep[:],
            cw.rearrange("e c -> (e c)")[None].to_broadcast((P, cfg.EPC * cfg.CAPP)),
        )

        # layer 2 for all experts
        for e in range(cfg.EPC):
            with ExitStack() as c2:
                m2pool = c2.enter_context(tc.tile_pool(name=f"p2m2{e}", bufs=2))
                tpool = c2.enter_context(tc.tile_pool(name=f"p2t{e}", bufs=3))
                kxm2_prod, kxm2_shape = _w_producer(m2pool, ew2[e], f"w2_{e}")

                def hg_kxn_producer(nc_, md, e=e):
                    return hg_tiles[e][md.n_tile_idx][
                        :, ts(md.k_tile_idx, md.k_subtiles), :
                    ]

                def l2_reducer(nc_, psum, sbuf, md, e=e):
                    # no b2: host passes cw/64, so psum*cwrep descales too.
                    # b2: host passes cw; descale psum explicitly, add b2.
                    do = (md.m_tile_idx * md.m_tile + md.m_subtile_idx * P) // P
                    n0 = md.n_tile_idx * md.n_tile + md.n_subtile_idx * md.n_subtile
                    if has_b2:
                        stage = tpool.tile([P, md.n_subtile], dt.float32, tag="stage")
                        nc_.scalar.activation(
                            stage[:], psum[:], ActFn.Copy, scale=1.0 / 64.0
                        )
                        nc_.vector.tensor_scalar_add(
                            stage[:], stage[:], b2_sb[:, e, do:do + 1]
                        )
                        src = stage
                    else:
                        src = psum
                    nc_.vector.tensor_mul(
                        sbuf[:], src[:], cwrep[:, e, ds(n0, md.n_subtile)]
                    )

                composable_matmul_tile_kernel(
                    tc=tc,
                    kxm_shape=kxm2_shape,
                    kxn_shape=hg_shape,
                    output_type=dt.bfloat16,
                    kxm_producer=kxm2_prod,
                    kxn_producer=hg_kxn_producer,
                    mxn_consumer=dma_to_dram_mxn(ygT[:, ds(e * cfg.CAP, cfg.CAP)]),
                    mxn_subtile_reducer=l2_reducer,
                    psum_n_bufs=2,
                )

    nc.compile()
    return nc


# --------------------------------------------------------------------------
# Phase 3: combine out = shared + y0 + y1
# --------------------------------------------------------------------------

def build_p3(cfg: Cfg, debug: bool = False, loop_n: int = 0):
    nc = bacc.Bacc("TRN2", target_bir_lowering=False, debug=debug)
    f32 = dt.float32
    aT = nc.dram_tensor("aT", [cfg.D, cfg.TPC], f32, kind="ExternalInput").ap()
    bT = nc.dram_tensor("bT", [cfg.D, cfg.TPC], dt.bfloat16, kind="ExternalInput").ap()
    cT = nc.dram_tensor("cT", [cfg.D, cfg.TPC], dt.bfloat16, kind="ExternalInput").ap()
    oT = nc.dram_tensor("oT", [cfg.D, cfg.TPC], f32, kind="ExternalOutput").ap()

    x_po = cfg.D // P
    CH = 128
    with tile.TileContext(nc) as tc, _maybe_loop(tc, loop_n), ExitStack() as ctx:
        pool = ctx.enter_context(tc.tile_pool(name="sb", bufs=3))
        aT_t, bT_t, cT_t, oT_t = _rearr2(aT), _rearr2(bT), _rearr2(cT), _rearr2(oT)
        for c in range(cfg.TPC // CH):
            a = pool.tile([P, x_po, CH], f32, tag="a")
            nc.sync.dma_start(a[:], aT_t[:, :, ts(c, CH)])
            b = pool.tile([P, x_po, CH], f32, tag="b")
            nc.gpsimd.dma_start(b[:], bT_t[:, :, ts(c, CH)])  # bf16 -> f32 cast
            cc = pool.tile([P, x_po, CH], f32, tag="c")
            nc.gpsimd.dma_start(cc[:], cT_t[:, :, ts(c, CH)])
            nc.vector.tensor_add(a[:], a[:], b[:])
            nc.vector.tensor_add(a[:], a[:], cc[:])
            nc.sync.dma_start(oT_t[:, :, ts(c, CH)], a[:])
    nc.compile()
    return nc


# --------------------------------------------------------------------------
# Host orchestration
# --------------------------------------------------------------------------

def _get(phase: str, cfg: Cfg, **bkw):
    key = (phase, cfg, tuple(sorted(bkw.items())))
    if key not in _cache:
        _cache[key] = {
            "p1": build_p1, "p2": build_p2, "p3": build_p3,
            "p1r": build_p1r, "p3s": build_p3s,
        }[phase](cfg, **bkw)
    return _cache[key]


CAPTURE: dict | None = None


def _run(phase: str, cfg: Cfg, in_maps, **bkw):
    if CAPTURE is not None:
        CAPTURE[phase] = (cfg, in_maps, bkw)
    nc = _get(phase, cfg, **bkw)
    r = run_bass_kernel_spmd(nc, in_maps, core_ids=list(range(cfg.n_cores)), trace=TRACE)
    LAST_EXEC_NS[phase] = r.exec_time_ns
    return r.results


def kernel(**inputs) -> np.ndarray:
    cfg = CFG
    x = np.ascontiguousarray(np.asarray(inputs["x"], dtype=np.float32))
    Bn, S, D = x.shape
    assert (Bn, S, D) == (cfg.n_cores, cfg.TPC, cfg.D)
    step_t = int(np.asarray(inputs["step_t"]))
    rw = np.ascontiguousarray(np.asarray(inputs["router_w"], np.float32)[step_t])
    re_w1 = np.ascontiguousarray(np.asarray(inputs["re_w1"], np.float32))
    re_b1 = np.ascontiguousarray(np.asarray(inputs["re_b1"], np.float32))
    re_w2 = np.ascontiguousarray(np.asarray(inputs["re_w2"], np.float32))
    re_b2 = np.ascontiguousarray(np.asarray(inputs["re_b2"], np.float32))
    sh_w1 = np.ascontiguousarray(np.asarray(inputs["sh_w1"], np.float32))
    sh_b1 = np.ascontiguousarray(np.asarray(inputs["sh_b1"], np.float32))
    sh_w2 = np.ascontiguousarray(np.asarray(inputs["sh_w2"], np.float32))
    sh_b2 = np.ascontiguousarray(np.asarray(inputs["sh_b2"], np.float32))

    xT = np.ascontiguousarray(x.transpose(0, 2, 1))  # [B, D, S] feature-major

    # ---- phase 1: router ----
    ident = np.eye(cfg.E, dtype=np.float32)
    in1 = [{"xT": xT[b], "rw": rw, "ident": ident} for b in range(cfg.n_cores)]
    r1 = _run("p1r", cfg, in1)

    idx = np.stack([r["ridx"][:, :2] for r in r1]).astype(np.int64)   # [B, S, 2]
    wts = np.stack([r["rwts"][:, :2] for r in r1])                    # [B, S, 2] f32

    T = Bn * S
    pair_e = idx.reshape(-1)                   # expert of pair p (p = g*2 + k)
    order = np.argsort(pair_e, kind="stable")  # pairs sorted by expert
    counts = np.bincount(pair_e, minlength=cfg.E)

    if counts.max() > cfg.CAP:  # safety net: regrow capacity, rebuild p2
        cfg = Cfg(CAP=int(-(-(counts.max() + 64) // P) * P))

    seg = np.zeros(cfg.E + 1, np.int64)
    seg[1:] = np.cumsum(counts)
    cols = np.zeros((cfg.E, cfg.CAP), np.int64)           # token col in xball
    cwarr = np.zeros((cfg.E, cfg.CAPP), np.float32)       # combine weights
    pos_of_pair = np.empty(2 * T, np.int64)
    wflat = wts.reshape(-1)
    for e in range(cfg.E):
        sl = order[seg[e]:seg[e + 1]]
        n = len(sl)
        cols[e, :n] = sl // 2
        cwarr[e, :n] = wflat[sl]
        pos_of_pair[sl] = np.arange(n)

    # ---- phase 2 (fp8: x/h at scale 1, weights at scale 64) ----
    has_b2 = bool(np.any(re_b2))
    F8 = ml_dtypes.float8_e4m3
    xf8 = np.ascontiguousarray(
        xT.transpose(1, 0, 2).reshape(cfg.D, -1)
    ).astype(F8)                                           # [D, T] fp8
    w1f8 = np.clip(re_w1 * 64.0, -240, 240).astype(F8)
    w2f8 = np.clip(re_w2 * 64.0, -240, 240).astype(F8)
    cw_pass = cwarr if has_b2 else cwarr * (1.0 / 64.0)
    in2 = []
    for c in range(cfg.n_cores):
        e0 = c * cfg.EPC
        xg = xf8[:, cols[e0:e0 + cfg.EPC].reshape(-1)]    # [D, EPC*CAP] fp8
        in2.append({
            "xgT": np.ascontiguousarray(xg),
            "ew1": np.ascontiguousarray(w1f8[e0:e0 + cfg.EPC]),
            "eb1": re_b1[e0:e0 + cfg.EPC],
            "ew2": np.ascontiguousarray(w2f8[e0:e0 + cfg.EPC]),
            "eb2": re_b2[e0:e0 + cfg.EPC],
            "cw": cw_pass[e0:e0 + cfg.EPC],
        })
    r2 = _run("p2", cfg, in2, has_b2=has_b2)

    # global y layout: expert e occupies columns [e*CAP, (e+1)*CAP)
    yall = np.concatenate([r["ygT"] for r in r2], axis=1)  # [D, E*CAP] bf16

    ycol_of_pair = pair_e * cfg.CAP + pos_of_pair          # [2T]
    ya = yall[:, ycol_of_pair[0::2]]                       # [D, T] slot k=0
    yb = yall[:, ycol_of_pair[1::2]]                       # [D, T] slot k=1

    # ---- phase 3: shared experts + combine ----
    sh_w1bf = np.ascontiguousarray(
        np.concatenate([sh_w1[0], sh_w1[1]], axis=1).astype(ml_dtypes.bfloat16)
    )
    sh_w2bf = sh_w2.astype(ml_dtypes.bfloat16)
    in3 = [
        {
            "xT": xT[b],
            "sw1": sh_w1bf, "sb1": sh_b1, "sw2": sh_w2bf, "sb2": sh_b2,
            "yaT": np.ascontiguousarray(ya[:, b * S:(b + 1) * S]),
            "ybT": np.ascontiguousarray(yb[:, b * S:(b + 1) * S]),
        }
        for b in range(cfg.n_cores)
    ]
    r3 = _run("p3s", cfg, in3, has_b2=bool(np.any(sh_b2)))

    out = np.stack([r["outT"] for r in r3])                # [B, D, S]
    return np.ascontiguousarray(out.transpose(0, 2, 1))    # [B, S, D] f32

